# revision 83
# baseline (speedup 1.0000x reference)
"""Trainium2 Bass kernel for causal multi-head attention with RoPE.

Problem: B=2, S=2048, D=1024, H=16 heads, L=64 head dim, causal, interleaved
RoPE, fp32 reference.

Sharding (8 cores): data-parallel over batch (2 groups of 4 cores) x tensor
parallel over heads (4 heads per core).  Each core:
  - computes Q^T/K^T (RoPE pair-split layout) and V for its 4 heads in a
    per-512-column pipeline (projection block -> V -> RoPE -> head-merge);
    the x^T block for st0 arrives as 8 per-dt chunk tiles so the first matmul
    starts as soon as one chunk + wq0 land,
  - flash-style causal attention with transposed scores [k, q]; softmax
    denominators ride along the PV matmul via a ones column appended to V;
    q-blocks 0-2 run "lite" (per-head 1-bank score tiles, scores pair-packed
    via tile_position) interleaved INTO the projection phase; q-block 3 runs
    wave-serial with 2-bank paired score tiles + batched exp,
  - per q-block: drains -> reciprocals -> DRAM-broadcast -> normalize ->
    8-way AllToAll.  The latency-long normalize/launch half (ship_b) is
    deferred one pipeline phase so its DMA latencies never head-of-line
    block an engine FIFO (which would cascade into rendezvous skew); attT
    gathers (which block their queue on the collective) are deferred until
    mid-body / right before the consuming out-projection.  qb3 ships per
    head-pair wave in two half-width A2As so wave 0's collective (and its
    rendezvous skew) hides under wave 1's attention,
  - out-projection of qb0 interleaves as PE gap-filler during qb3's
    attention; qb1's and qb2's are pinned into the wave-0 / wave-1 ship
    windows; qb3's runs after the final half-gather.
Host glue: shard/permute/cast inputs, scatter the row-sharded outputs back,
add the output-projection bias.
"""

import sys

import numpy as np

for _p in ("/opt/trn_rl_repo",):
    if _p not in sys.path:
        sys.path.insert(0, _p)

import ml_dtypes

import concourse.bass as bass  # noqa: F401  (registers types)
import concourse.mybir as mybir
import concourse.tile as tile
from concourse import bacc
from concourse import bass_utils

BF16 = mybir.dt.bfloat16
F32 = mybir.dt.float32
NPBF16 = ml_dtypes.bfloat16
AF = mybir.ActivationFunctionType
ALU = mybir.AluOpType

B, S, D = 2, 2048, 1024
H, L = 16, 64
HPC = 4  # heads per core
N_CORES = 8
QB = 512  # query block (columns of transposed scores)
NQB = S // QB  # 4
NKT = S // 128  # 16 key tiles
ROPE_BASE = 10000.0
A2A_GROUP = [list(range(N_CORES))]


def build_program():
    nc = bacc.Bacc(
        "TRN2", target_bir_lowering=False, debug=False, num_devices=N_CORES
    )

    # ---- I/O ----
    xts_d = nc.dram_tensor("xts", [4 * 128, 8 * 512], BF16, kind="ExternalInput")
    wq0_d = nc.dram_tensor("wq0", [128, 8 * 128], BF16, kind="ExternalInput")
    wq1_d = nc.dram_tensor("wq1", [128, 8 * 128], BF16, kind="ExternalInput")
    wk0_d = nc.dram_tensor("wk0", [128, 8 * 128], BF16, kind="ExternalInput")
    wk1_d = nc.dram_tensor("wk1", [128, 8 * 128], BF16, kind="ExternalInput")
    wv_d = nc.dram_tensor("wv", [128, 8 * 256], BF16, kind="ExternalInput")
    wo_d = nc.dram_tensor("wo", [128, 8 * D], BF16, kind="ExternalInput")
    bq0_d = nc.dram_tensor("bq0", [128, 1], F32, kind="ExternalInput")
    bq1_d = nc.dram_tensor("bq1", [128, 1], F32, kind="ExternalInput")
    bk0_d = nc.dram_tensor("bk0", [128, 1], F32, kind="ExternalInput")
    bk1_d = nc.dram_tensor("bk1", [128, 1], F32, kind="ExternalInput")
    bvr_d = nc.dram_tensor("bvr", [1, 256], BF16, kind="ExternalInput")
    cos_d = nc.dram_tensor("cos32", [32, S], BF16, kind="ExternalInput")
    sin_d = nc.dram_tensor("sin32", [32, S], BF16, kind="ExternalInput")
    tri_d = nc.dram_tensor("tri", [128, 128], BF16, kind="ExternalInput")
    out_d = nc.dram_tensor("out", [NQB * 128, D], BF16, kind="ExternalOutput")

    recip_d = nc.dram_tensor("recipd", [4 * HPC, 512], BF16, kind="Internal")
    a2ain_d = [
        nc.dram_tensor(f"a2ain{qb}", [N_CORES * 128, 128], BF16, kind="Internal")
        for qb in range(3)
    ]
    a2aout_d = [
        nc.dram_tensor(f"a2aout{qb}", [N_CORES * 128, 128], BF16, kind="Internal")
        for qb in range(3)
    ]
    # qb3 ships per head-pair wave in two half-width A2As: wave 0's
    # collective (and its rendezvous skew) hides under wave 1's attention
    a2ain3_d = [
        nc.dram_tensor(f"a2ain3{w}", [N_CORES * 128, 64], BF16, kind="Internal")
        for w in range(2)
    ]
    a2aout3_d = [
        nc.dram_tensor(f"a2aout3{w}", [N_CORES * 128, 64], BF16, kind="Internal")
        for w in range(2)
    ]

    with tile.TileContext(nc) as tc:
        with (
            tc.tile_pool(name="const", bufs=1) as cpool,
            tc.tile_pool(name="xp", bufs=1) as xpool,
            tc.tile_pool(name="qk", bufs=1) as qkpool,
            tc.tile_pool(name="rtmp", bufs=2) as rtmp,
            tc.tile_pool(name="ptp", bufs=3) as ptpool,
            tc.tile_pool(name="att", bufs=1) as attpool,
            tc.tile_pool(name="bc", bufs=2) as bcpool,
            tc.tile_pool(name="osb", bufs=2) as opool,
            tc.tile_pool(name="a4p", bufs=4) as a4pool,
        ):
            # ---- load order tuned for earliest first matmul ----
            # wq0 + per-dt x chunks first so the q0/st0 matmul stream starts
            # as soon as ~1 chunk lands; small constants ride the scalar /
            # gpsimd DGEs so they don't serialize behind the bulk loads on
            # the sync DGE.
            def load_w(dram, cols):
                t = cpool.tile([128, 8, cols], BF16, tag=f"w_{dram.name}")
                nc.sync.dma_start(
                    t[:].rearrange("p a b -> p (a b)"), dram.ap()
                )
                return t

            def load_c(dram, shape, dt, tag, eng):
                t = cpool.tile(shape, dt, tag=tag)
                eng.dma_start(t[:], dram.ap())
                return t

            xts_r = xts_d.ap().rearrange(
                "(st p) (dt s) -> st p dt s", st=4, dt=8
            )

            wq0_sb = load_w(wq0_d, 128)
            x0d = [
                xpool.tile([128, 512], BF16, tag=f"x0d{dt}", name=f"x0d{dt}")
                for dt in range(8)
            ]
            for dt in range(8):
                nc.sync.dma_start(x0d[dt][:], xts_r[0][:, dt, :])
            wq1_sb = load_w(wq1_d, 128)
            wk0_sb = load_w(wk0_d, 128)
            wk1_sb = load_w(wk1_d, 128)
            xt1_sb = xpool.tile([128, 8, 512], BF16, tag="xt1")
            nc.sync.dma_start(xt1_sb[:], xts_r[1])
            wv_sb = load_w(wv_d, 256)
            xt2_sb = xpool.tile([128, 8, 512], BF16, tag="xt2")
            nc.sync.dma_start(xt2_sb[:], xts_r[2])
            xt3_sb = xpool.tile([128, 8, 512], BF16, tag="xt3")
            nc.sync.dma_start(xt3_sb[:], xts_r[3])

            bq0_sb = load_c(bq0_d, [128, 1], F32, "bq0", nc.scalar)
            bq1_sb = load_c(bq1_d, [128, 1], F32, "bq1", nc.scalar)
            bk0_sb = load_c(bk0_d, [128, 1], F32, "bk0", nc.scalar)
            bk1_sb = load_c(bk1_d, [128, 1], F32, "bk1", nc.scalar)
            # v-bias broadcast across all partitions: the bias-add rides the
            # V PSUM drain on DVE instead of a rank-1 matmul per block
            bvr_sb = cpool.tile([128, 4, 64], BF16, tag="bvr")
            nc.scalar.dma_start(
                bvr_sb[:].rearrange("p a b -> p (a b)"),
                bvr_d.ap().to_broadcast((128, 256)),
            )
            tri_sb = load_c(tri_d, [128, 128], BF16, "tri", nc.gpsimd)

            cos_sb = cpool.tile([128, S], BF16, tag="cos4")
            sin_sb = cpool.tile([128, S], BF16, tag="sin4")
            nc.gpsimd.dma_start(cos_sb[0:32, :], cos_d.ap())
            nc.gpsimd.dma_start(sin_sb[0:32, :], sin_d.ap())
            nc.gpsimd.dma_start(cos_sb[32:64, :], cos_sb[0:32, :])
            nc.gpsimd.dma_start(sin_sb[32:64, :], sin_sb[0:32, :])
            nc.gpsimd.dma_start(cos_sb[64:128, :], cos_sb[0:64, :])
            nc.gpsimd.dma_start(sin_sb[64:128, :], sin_sb[0:64, :])

            def xt_ap(st, dt):
                """x^T slice for (st, dt): [128, 512] local columns."""
                if st == 0:
                    return x0d[dt][:]
                return (xt1_sb, xt2_sb, xt3_sb)[st - 1][:, dt, :]

            # ---- persistent SBUF state ----
            q0_sb = qkpool.tile([128, S], BF16, tag="q0")
            q1_sb = qkpool.tile([128, S], BF16, tag="q1")
            k0_sb = qkpool.tile([128, S], BF16, tag="k0")
            k1_sb = qkpool.tile([128, S], BF16, tag="k1")
            v_sb = qkpool.tile([128, NKT, HPC * 65], BF16, tag="v")
            nc.vector.memset(
                v_sb[:].rearrange("p t (h c) -> p t h c", c=65)[:, :, :, 64:65], 1.0
            )
            qm = [
                qkpool.tile([128, S], BF16, tag=f"qm{w}", name=f"qm{w}")
                for w in range(2)
            ]
            km = [
                qkpool.tile([128, S], BF16, tag=f"km{w}", name=f"km{w}")
                for w in range(2)
            ]

            sums_sb = attpool.tile([128, 64], F32, tag="sums")
            recip_sb = attpool.tile([128, 64], BF16, tag="recip")
            tri_b2 = tri_sb[:, None, :].to_broadcast((128, 2, 128))

            attTs = []
            last_pv = {}

            # ---- helpers ----
            def proj_st(projp, st):
                sl = slice(st * 512, (st + 1) * 512)
                for dst, w_sb, b_sb in (
                    (q0_sb, wq0_sb, bq0_sb),
                    (q1_sb, wq1_sb, bq1_sb),
                    (k0_sb, wk0_sb, bk0_sb),
                    (k1_sb, wk1_sb, bk1_sb),
                ):
                    ps = projp.tile([128, 512], F32, tag="pq")
                    for dt_ in range(8):
                        nc.tensor.matmul(
                            ps[:],
                            w_sb[:, dt_, :],
                            xt_ap(st, dt_),
                            start=(dt_ == 0),
                            stop=(dt_ == 7),
                        )
                    nc.vector.tensor_scalar(
                        dst[:, sl], ps[:], b_sb[:, 0:1], None, ALU.add
                    )
                if st == 0:
                    warm_act = cpool.tile([128, 1], F32, tag="warm_act")
                    nc.scalar.activation(warm_act[:], bq0_sb[:], AF.Exp)
                for sub in range(4):
                    stv = 4 * st + sub
                    ps = projp.tile([128, 256], F32, tag="pvj")
                    for dt_ in range(8):
                        nc.tensor.matmul(
                            ps[:],
                            xt_ap(st, dt_)[:, sub * 128 : (sub + 1) * 128],
                            wv_sb[:, dt_, :],
                            start=(dt_ == 0),
                            stop=(dt_ == 7),
                        )
                    nc.vector.tensor_tensor(
                        v_sb[:, stv, :].rearrange("p (h c) -> p h c", c=65)[
                            :, :, 0:64
                        ],
                        ps[:].rearrange("p (h c) -> p h c", c=64),
                        bvr_sb[:],
                        ALU.add,
                    )
                # RoPE for this st (DVE)
                for x0, x1 in ((q0_sb, q1_sb), (k0_sb, k1_sb)):
                    m1 = rtmp.tile([128, 512], BF16, tag="m1")
                    m2 = rtmp.tile([128, 512], BF16, tag="m2")
                    m3 = rtmp.tile([128, 512], BF16, tag="m3")
                    m4 = rtmp.tile([128, 512], BF16, tag="m4")
                    nc.vector.tensor_tensor(m1[:], x0[:, sl], cos_sb[:, sl], ALU.mult)
                    nc.vector.tensor_tensor(m2[:], x1[:, sl], sin_sb[:, sl], ALU.mult)
                    nc.vector.tensor_tensor(m3[:], x0[:, sl], sin_sb[:, sl], ALU.mult)
                    nc.vector.tensor_tensor(m4[:], x1[:, sl], cos_sb[:, sl], ALU.mult)
                    nc.vector.tensor_tensor(x0[:, sl], m1[:], m2[:], ALU.subtract)
                    nc.vector.tensor_tensor(x1[:, sl], m3[:], m4[:], ALU.add)
                # merge RoPE'd halves into per-head-contiguous layouts.
                # All merges ride the sync DGE: the gpsimd queue blocks on
                # each collective's completion, and k-merges queued behind a
                # trigger would stall the next q-block's scores on the
                # previous A2A's rendezvous skew.  The first scores of a
                # block only need the first ~4 merges, so the serial chain
                # is demand-paced.
                for w in range(2):
                    for hh in range(2):
                        h = 2 * w + hh
                        nc.sync.dma_start(
                            qm[w][64 * hh : 64 * hh + 32, sl],
                            q0_sb[32 * h : 32 * h + 32, sl],
                        )
                        nc.sync.dma_start(
                            qm[w][64 * hh + 32 : 64 * hh + 64, sl],
                            q1_sb[32 * h : 32 * h + 32, sl],
                        )
                        nc.sync.dma_start(
                            km[w][64 * hh : 64 * hh + 32, sl],
                            k0_sb[32 * h : 32 * h + 32, sl],
                        )
                        nc.sync.dma_start(
                            km[w][64 * hh + 32 : 64 * hh + 64, sl],
                            k1_sb[32 * h : 32 * h + 32, sl],
                        )

            def drain_pair(stg, att4, w, pvl):
                """Copy a head-pair's denominator rows + attended blocks out
                of PSUM into the staging tiles (releases the PSUM banks)."""
                nc.vector.tensor_copy(
                    stg[64:65, 2 * w : 2 * w + 2, :], pvl[64:65, :, :]
                )
                nc.vector.tensor_copy(
                    att4[:, 2 * w : 2 * w + 2, :], pvl[0:64, :, :]
                )

            ship_st = {}
            ship_sums = {}

            def ship_a(qb, stg, att4):
                """Ship stage A (emitted right after qb's drains):
                denominators -> reciprocals -> DRAM -> broadcast loads."""
                ship_sums[qb] = nc.sync.dma_start(
                    sums_sb[32 * qb : 32 * qb + 32, :],
                    stg[64:65, :, :],
                )
                with nc.allow_low_precision(
                    reason="bf16 recip matches the prior rb-cast path"
                ):
                    nc.vector.reciprocal(
                        recip_sb[32 * qb : 32 * qb + 32, :],
                        sums_sb[32 * qb : 32 * qb + 32, :],
                    )
                nc.sync.dma_start(
                    recip_d[4 * qb : 4 * qb + 4, :],
                    recip_sb[32 * qb : 32 * qb + 32, :],
                )
                # all broadcast loads on sync: the gpsimd queue blocks on the
                # previous collective's completion, so anything here routed
                # through it inherits that collective's rendezvous skew
                bct4 = bcpool.tile([64, HPC, 512], BF16, tag="bct4",
                                   name=f"bct4_{qb}")
                for h in range(HPC):
                    nc.sync.dma_start(
                        bct4[:, h, :],
                        recip_d[4 * qb + h : 4 * qb + h + 1, :]
                        .to_broadcast((64, 512)),
                    )
                ship_st[qb] = (att4, bct4)

            def ship_b(qb, pin_after=None):
                """Ship stage B: normalize, scatter into the A2A layout, and
                launch the collective.  Deferred one pipeline phase after
                ship_a so each engine reaches these ops with the inputs long
                since ready -- a stage-A DMA latency here would head-of-line
                block the engine FIFOs and cascade into rendezvous skew.
                (Gathers from a2aout are deferred even further, right before
                the consuming out-projection.)"""
                att4, bct4 = ship_st.pop(qb)
                a2a_w = a2ain_d[qb].ap().rearrange(
                    "(j hp c) (t r) -> hp t c j r", j=8, hp=2, c=64, t=2
                )
                for t in range(2):
                    sl_ = att4[:, 2 * t : 2 * t + 2, :].rearrange(
                        "c h r -> c (h r)"
                    )
                    nc.vector.tensor_tensor(
                        sl_,
                        sl_,
                        bct4[:, 2 * t : 2 * t + 2, :].rearrange(
                            "c h r -> c (h r)"
                        ),
                        ALU.mult,
                    )
                    for i in range(2):
                        h = 2 * t + i
                        nc.sync.dma_start(
                            a2a_w[h % 2, h // 2],
                            att4[:, h, :].rearrange("c (j r) -> c j r", j=8),
                        )
                cc = nc.gpsimd.collective_compute(
                    "AllToAll",
                    ALU.bypass,
                    replica_groups=A2A_GROUP,
                    ins=[a2ain_d[qb][:]],
                    outs=[a2aout_d[qb][:]],
                )
                if pin_after is not None:
                    # the collective blocks its queue until completion; make
                    # sure earlier gathers cannot be scheduled behind it
                    tile.add_dep_helper(
                        cc.ins, pin_after.ins, sync=True,
                        reason="collective after prior gathers",
                    )

            ship3_sums = {}

            def ship3_wave(w, stg, att4):
                """qb3 ships per wave: sums -> recip -> broadcast -> norm ->
                half-width A2A over this head-pair's columns.  Wave 0's
                collective runs under wave 1's attention; only wave 1's is
                tail-exposed."""
                qb = 3
                # 32-aligned 32x32 staging region per wave (wave 1 reuses
                # qb0's long-retired rows)
                r0 = 96 if w == 0 else 0
                ship3_sums[w] = nc.sync.dma_start(
                    sums_sb[r0 : r0 + 32, 0:32],
                    stg[64:65, 2 * w : 2 * w + 2, :],
                )
                with nc.allow_low_precision(
                    reason="bf16 recip matches the prior rb-cast path"
                ):
                    nc.vector.reciprocal(
                        recip_sb[r0 : r0 + 32, 0:32],
                        sums_sb[r0 : r0 + 32, 0:32],
                    )
                nc.sync.dma_start(
                    recip_d[4 * qb + 2 * w : 4 * qb + 2 * w + 2, :],
                    recip_sb[r0 : r0 + 32, 0:32],
                )
                bct2 = bcpool.tile([64, 2, 512], BF16, tag="bct2",
                                   name=f"bct2_{w}")
                for i in range(2):
                    nc.sync.dma_start(
                        bct2[:, i, :],
                        recip_d[4 * qb + 2 * w + i : 4 * qb + 2 * w + i + 1, :]
                        .to_broadcast((64, 512)),
                    )
                sl_ = att4[:, 2 * w : 2 * w + 2, :].rearrange(
                    "c h r -> c (h r)"
                )
                nc.vector.tensor_tensor(
                    sl_, sl_, bct2[:].rearrange("c h r -> c (h r)"), ALU.mult
                )
                a2a_w = a2ain3_d[w].ap().rearrange(
                    "(j hp c) r -> hp c j r", j=8, hp=2, c=64
                )
                for i in range(2):
                    nc.sync.dma_start(
                        a2a_w[i],
                        att4[:, 2 * w + i, :].rearrange(
                            "c (j r) -> c j r", j=8
                        ),
                    )
                nc.gpsimd.collective_compute(
                    "AllToAll",
                    ALU.bypass,
                    replica_groups=A2A_GROUP,
                    ins=[a2ain3_d[w][:]],
                    outs=[a2aout3_d[w][:]],
                )

            def gather3_wave(w, attT):
                """Gather wave w's redistributed rows into attT slots
                4w..4w+4.  Wave 0: gpsimd only (it is blocked on the wave-0
                collective anyway, hidden under wave 1).  Wave 1: gpsimd +
                scalar (both idle in the tail)."""
                srcr = a2aout3_d[w].ap().rearrange(
                    "(i p) r -> p i r", p=128
                )
                for bh in range(2):
                    eng = nc.gpsimd if w == 0 else (nc.gpsimd, nc.scalar)[bh]
                    eng.dma_start(
                        attT[:, 4 * w : 4 * w + 4, 64 * bh : 64 * bh + 64],
                        srcr[:, 4 * bh : 4 * bh + 4, :],
                    )

            def gather_attT(qb, tail=False):
                """Pull this q-block's redistributed attended rows out of the
                A2A output.  These wait on the collective, so they must never
                sit ahead of other pending work in a DGE queue; the final
                gather goes gpsimd-only so the sync queue (y stores) never
                blocks on the last collective."""
                attT = a4pool.tile([128, 8, 128], BF16, tag="attT",
                                   name=f"attT_{qb}")
                srcr = a2aout_d[qb].ap().rearrange(
                    "(i p) (t r) -> p i t r", p=128, t=2
                )
                last = None
                for t in range(2):
                    for bh in range(2):
                        # tail: scalar's queue is empty by now, so blocking
                        # it on the last collective is free parallelism
                        eng = (nc.gpsimd, nc.scalar)[bh] if tail else (
                            (nc.gpsimd, nc.sync)[bh]
                        )
                        last = eng.dma_start(
                            attT[:, 4 * t : 4 * t + 4, 64 * bh : 64 * bh + 64],
                            srcr[:, 4 * bh : 4 * bh + 4, t],
                        )
                attTs.append(attT)
                return last

            def qb_lite(plite, qb, psc_bufs=2, paired=False, ship3=False):
                """Attention for qb in two head-pair passes (runs interleaved
                with the projection phase).  paired=True packs the two heads
                of a wave into one 2-bank psc tile: the scores matmuls run
                concurrently via tile_position row strips and the exp is one
                batched ACT call -- needs 4+2 banks, only affordable for the
                standalone qb3 block."""
                stg = bcpool.tile([65, HPC, 512], F32, tag="stg",
                                  name=f"stg_{qb}")
                att4 = bcpool.tile([64, HPC, 512], BF16, tag="att4",
                                   name=f"att4_{qb}")
                nkt = 4 * qb + 4
                for w in range(2):
                    pvl = plite.tile([65, 2, 512], F32, tag="pvl",
                                     name=f"pvl_{qb}_{w}")
                    for kt in range(nkt):
                        j = kt - 4 * qb
                        qlo = max(0, j * 128)
                        g0 = qb * 512 + qlo
                        g1 = (qb + 1) * 512
                        if paired:
                            psc = plite.tile(
                                [128, 2, 512], F32, tag="psc2",
                                bufs=psc_bufs,
                                name=f"psc2_{qb}_{w}_{kt}",
                            )
                            for hh in range(2):
                                nc.tensor.matmul(
                                    psc[:, hh, qlo:512],
                                    km[w][64 * hh : 64 * hh + 64,
                                          kt * 128 : (kt + 1) * 128],
                                    qm[w][64 * hh : 64 * hh + 64, g0:g1],
                                    start=True,
                                    stop=True,
                                    tile_position=(64 * hh, 0),
                                )
                            pt = ptpool.tile(
                                [128, 2, 512], BF16, tag="ptp2",
                                name=f"ptp2_{qb}_{w}_{kt}",
                            )
                            if qlo == 0:
                                nc.scalar.activation(
                                    pt[:].rearrange("p a b -> p (a b)"),
                                    psc[:].rearrange("p a b -> p (a b)"),
                                    AF.Exp, scale=0.125,
                                )
                            else:
                                nc.scalar.activation(
                                    pt[:, :, qlo:512], psc[:, :, qlo:512],
                                    AF.Exp, scale=0.125,
                                )
                            if g0 == kt * 128:
                                nc.vector.tensor_tensor(
                                    pt[:, :, qlo : qlo + 128],
                                    pt[:, :, qlo : qlo + 128],
                                    tri_b2,
                                    ALU.mult,
                                )
                            for hh in range(2):
                                h = 2 * w + hh
                                mm = nc.tensor.matmul(
                                    pvl[:, hh, qlo:512],
                                    v_sb[:, kt, 65 * h : 65 * h + 65],
                                    pt[:, hh, qlo:512],
                                    start=(kt == 0),
                                    stop=(kt == nkt - 1),
                                )
                                last_pv[qb] = mm
                            continue
                        # both heads' scores emitted adjacently: different
                        # row strips + different PSUM banks, so the PE packs
                        # them concurrently (second MM is ~4ns)
                        pscs = []
                        for hh in range(2):
                            psc = plite.tile(
                                [128, 512], F32, tag="pscl", bufs=psc_bufs,
                                name=f"pscl_{qb}_{w}_{kt}_{hh}",
                            )
                            nc.tensor.matmul(
                                psc[:, qlo:512],
                                km[w][64 * hh : 64 * hh + 64,
                                      kt * 128 : (kt + 1) * 128],
                                qm[w][64 * hh : 64 * hh + 64, g0:g1],
                                start=True,
                                stop=True,
                                tile_position=(64 * hh, 0),
                            )
                            pscs.append(psc)
                        for hh in range(2):
                            h = 2 * w + hh
                            pt = ptpool.tile(
                                [128, 512], BF16, tag="ptl",
                                name=f"ptl_{qb}_{w}_{kt}_{hh}",
                            )
                            nc.scalar.activation(
                                pt[:, qlo:512], pscs[hh][:, qlo:512],
                                AF.Exp, scale=0.125,
                            )
                            if g0 == kt * 128:
                                nc.vector.tensor_tensor(
                                    pt[:, qlo : qlo + 128],
                                    pt[:, qlo : qlo + 128],
                                    tri_sb[:],
                                    ALU.mult,
                                )
                            mm = nc.tensor.matmul(
                                pvl[:, hh, qlo:512],
                                v_sb[:, kt, 65 * h : 65 * h + 65],
                                pt[:, qlo:512],
                                start=(kt == 0),
                                stop=(kt == nkt - 1),
                            )
                            last_pv[qb] = mm
                    drain_pair(stg, att4, w, pvl)
                    if ship3:
                        ship3_wave(w, stg, att4)
                        gather3_wave(w, attT3)
                if not ship3:
                    ship_a(qb, stg, att4)

            # ---- projection phase with qb0-qb2 lite attention woven in ----
            with (
                tc.tile_pool(name="projp", bufs=2, space="PSUM") as projp,
                tc.tile_pool(name="plite", bufs=1, space="PSUM") as plite,
            ):
                proj_st(projp, 0)
                qb_lite(plite, 0)
                proj_st(projp, 1)
                ship_b(0)
                qb_lite(plite, 1)
                proj_st(projp, 2)
                ship_b(1)
                qb_lite(plite, 2)
                proj_st(projp, 3)
                ship_b(2)
                # gathers for qb0-2 land here, mid-body: their collectives
                # complete during the projection phases, so these never
                # block, and attT0-2 are ready before the qb3 filler window
                for qb in range(3):
                    gather_attT(qb)

            # wo arrives during attention; needed only for the tail out-proj
            wo_sb = cpool.tile([128, 8, D], BF16)
            nc.sync.dma_start(wo_sb[:].rearrange("p a b -> p (a b)"), wo_d.ap())

            # ---- full-width attention for qb3, with the out-projections of
            # qb0-2 emitted after it as lower-priority PE gap-filler: they
            # hide the exp->PV chain latency, keep HAM warm through qb3's
            # ship window, and leave only qb3's own out-proj in the tail.
            def outproj(opsum, qb, after=None):
                attT = attTs[qb]
                y_sb = opool.tile([128, D], BF16, tag="y", name=f"y_{qb}")
                last = None
                for dc in range(2):
                    po = opsum.tile([128, 512], F32, tag="po",
                                    name=f"po_{qb}_{dc}")
                    for s in range(8):
                        mm = nc.tensor.matmul(
                            po[:],
                            attT[:, s, :],
                            wo_sb[:, s, dc * 512 : (dc + 1) * 512],
                            start=(s == 0),
                            stop=(s == 7),
                        )
                        last = mm
                        if after is not None and dc == 0 and s == 0:
                            # explicit ordering handle (scheduler otherwise
                            # reorders these across the ship3 window)
                            tile.add_dep_helper(
                                mm.ins, after.ins, sync=True,
                                reason="outproj ordering across ship3",
                            )
                    nc.vector.tensor_copy(
                        y_sb[:, dc * 512 : (dc + 1) * 512], po[:]
                    )
                nc.sync.dma_start(out_d[qb * 128 : (qb + 1) * 128, :], y_sb[:])
                return last

            # qb3 runs lite-style with a deep psc rotation (4 banks) so the
            # scores stream can run ahead of the ACT exp pacing; out-proj of
            # qb0/1 interleaves as PE filler during qb3's attention, out-proj
            # of qb2 is held back to bridge the ship3/A2A window, and
            # out-proj 3 is pinned after it so the scheduler cannot park
            # out-proj 2 behind the gather-gated final block.
            with (
                tc.tile_pool(name="pl3", bufs=1, space="PSUM") as pl3,
                tc.tile_pool(name="op", bufs=2, space="PSUM") as opsum,
            ):
                attT3 = a4pool.tile([128, 8, 128], BF16, tag="attT",
                                    name="attT_3")
                attTs.append(attT3)
                qb_lite(pl3, 3, psc_bufs=2, paired=True, ship3=True)
                outproj(opsum, 0, after=ship3_sums[0])
                outproj(opsum, 1, after=ship3_sums[1])
                op2_last = outproj(opsum, 2, after=ship3_sums[1])
                outproj(opsum, 3, after=op2_last)

    nc.compile()
    return nc


def make_in_maps(x, Wq, bq, Wk, bk, Wv, bv, Wo):
    inv = 1.0 / (ROPE_BASE ** (2.0 * np.arange(32, dtype=np.float64) / L))
    ang = np.arange(S, dtype=np.float64)[:, None] * inv[None, :]  # [S, 32]
    cos32 = np.ascontiguousarray(np.cos(ang).T).astype(NPBF16)  # [32, S]
    sin32 = np.ascontiguousarray(np.sin(ang).T).astype(NPBF16)
    tri = (np.arange(128)[None, :] >= np.arange(128)[:, None]).astype(NPBF16)

    # Wo rows permuted to the attT channel order: slot s = 4*t + gi, row
    # s*128+p holds Wo[64*(4*gi + 2*t + (p>=64)) + p%64]  (same for all cores).
    perm = np.empty(D, np.int64)
    for s_ in range(8):
        t, gi = divmod(s_, 4)
        for p in range(128):
            h = 4 * gi + 2 * t + (1 if p >= 64 else 0)
            perm[s_ * 128 + p] = 64 * h + (p % 64)
    wo_perm = np.ascontiguousarray(Wo[perm, :]).astype(NPBF16)
    wo_pm = np.ascontiguousarray(
        wo_perm.reshape(8, 128, D).transpose(1, 0, 2).reshape(128, 8 * D)
    )

    def _pm(w):
        # [1024, C] -> partition-major [128, 8*C] bf16 (contiguous lines)
        w = np.asarray(w).astype(NPBF16)
        c = w.shape[1]
        return np.ascontiguousarray(
            w.reshape(8, 128, c).transpose(1, 0, 2).reshape(128, 8 * c)
        )

    in_maps = []
    for c in range(N_CORES):
        b, g = divmod(c, HPC)
        even = np.concatenate([64 * h + 2 * np.arange(32) for h in range(4 * g, 4 * g + 4)])
        odd = even + 1
        vcols = np.arange(256 * g, 256 * (g + 1))
        xt = np.ascontiguousarray(x[b].T).astype(NPBF16)  # [D, S]
        xts = np.ascontiguousarray(
            xt.reshape(8, 128, 4, 512).transpose(2, 1, 0, 3).reshape(512, 4096)
        )
        in_maps.append(
            {
                "xts": xts,
                "wq0": _pm(Wq[:, even]),
                "wq1": _pm(Wq[:, odd]),
                "wk0": _pm(Wk[:, even]),
                "wk1": _pm(Wk[:, odd]),
                "wv": _pm(Wv[:, vcols]),
                "wo": wo_pm,
                "bq0": bq[even].reshape(128, 1).astype(np.float32),
                "bq1": bq[odd].reshape(128, 1).astype(np.float32),
                "bk0": bk[even].reshape(128, 1).astype(np.float32),
                "bk1": bk[odd].reshape(128, 1).astype(np.float32),
                "bvr": bv[vcols].reshape(1, 256).astype(NPBF16),
                "cos32": cos32,
                "sin32": sin32,
                "tri": tri,
            }
        )
    return in_maps


def assemble_output(results, bo):
    out = np.empty((B, S, D), np.float32)
    for c in range(N_CORES):
        sh = np.asarray(results[c]["out"]).astype(np.float32).reshape(NQB, 128, D)
        for qb in range(NQB):
            r0 = qb * 512 + c * 64
            out[0, r0 : r0 + 64, :] = sh[qb][0:64]
            out[1, r0 : r0 + 64, :] = sh[qb][64:128]
    out += bo[None, None, :].astype(np.float32)
    return out


_CACHE = {}


def kernel(x, Wq, bq, Wk, bk, Wv, bv, Wo, bo, **run_kwargs):
    if "nc" not in _CACHE:
        _CACHE["nc"] = build_program()
    nc = _CACHE["nc"]
    in_maps = make_in_maps(
        np.asarray(x), np.asarray(Wq), np.asarray(bq), np.asarray(Wk),
        np.asarray(bk), np.asarray(Wv), np.asarray(bv), np.asarray(Wo),
    )
    res = bass_utils.run_bass_kernel_spmd(
        nc, in_maps, core_ids=list(range(N_CORES)), **run_kwargs
    )
    out = assemble_output(res.results, np.asarray(bo))
    kernel.last_results = res
    return out



# revision 84
# speedup vs baseline: 1.2224x; 1.2224x over previous
"""Trainium2 Bass kernel for causal multi-head attention with RoPE.

Problem: B=2, S=2048, D=1024, H=16 heads, L=64 head dim, causal, interleaved
RoPE, fp32 reference.

Sharding (8 cores): data-parallel over batch (2 groups of 4 cores) x tensor
parallel over heads (4 heads per core).  Each core:
  - computes Q^T/K^T (RoPE pair-split layout) and V for its 4 heads in a
    per-512-column pipeline (projection block -> V -> RoPE -> head-merge);
    the x^T block for st0 arrives as 8 per-dt chunk tiles so the first matmul
    starts as soon as one chunk + wq0 land,
  - flash-style causal attention with transposed scores [k, q]; softmax
    denominators ride along the PV matmul via a ones column appended to V;
    q-blocks 0-2 run "lite" (per-head 1-bank score tiles, scores pair-packed
    via tile_position) interleaved INTO the projection phase; q-block 3 runs
    wave-serial with 2-bank paired score tiles + batched exp,
  - per q-block: drains -> reciprocals -> DRAM-broadcast -> normalize ->
    8-way AllToAll.  The latency-long normalize/launch half (ship_b) is
    deferred one pipeline phase so its DMA latencies never head-of-line
    block an engine FIFO (which would cascade into rendezvous skew); attT
    gathers (which block their queue on the collective) are deferred until
    mid-body / right before the consuming out-projection.  qb3 ships per
    head-pair wave in two half-width A2As so wave 0's collective (and its
    rendezvous skew) hides under wave 1's attention,
  - out-projection of qb0 interleaves as PE gap-filler during qb3's
    attention; qb1's and qb2's are pinned into the wave-0 / wave-1 ship
    windows; qb3's runs after the final half-gather.
Host glue: shard/permute/cast inputs, scatter the row-sharded outputs back,
add the output-projection bias.
"""

import sys

import numpy as np

for _p in ("/opt/trn_rl_repo",):
    if _p not in sys.path:
        sys.path.insert(0, _p)

import ml_dtypes

import concourse.bass as bass  # noqa: F401  (registers types)
import concourse.mybir as mybir
import concourse.tile as tile
from concourse import bacc
from concourse import bass_utils

BF16 = mybir.dt.bfloat16
F32 = mybir.dt.float32
NPBF16 = ml_dtypes.bfloat16
AF = mybir.ActivationFunctionType
ALU = mybir.AluOpType

B, S, D = 2, 2048, 1024
H, L = 16, 64
HPC = 4  # heads per core
N_CORES = 8
QB = 512  # query block (columns of transposed scores)
NQB = S // QB  # 4
NKT = S // 128  # 16 key tiles
ROPE_BASE = 10000.0
A2A_GROUP = [list(range(N_CORES))]


def build_program():
    nc = bacc.Bacc(
        "TRN2", target_bir_lowering=False, debug=False, num_devices=N_CORES
    )

    # ---- I/O ----
    xts_d = nc.dram_tensor("xts", [4 * 128, 8 * 512], BF16, kind="ExternalInput")
    wq0_d = nc.dram_tensor("wq0", [128, 8 * 128], BF16, kind="ExternalInput")
    wq1_d = nc.dram_tensor("wq1", [128, 8 * 128], BF16, kind="ExternalInput")
    wk0_d = nc.dram_tensor("wk0", [128, 8 * 128], BF16, kind="ExternalInput")
    wk1_d = nc.dram_tensor("wk1", [128, 8 * 128], BF16, kind="ExternalInput")
    wv_d = nc.dram_tensor("wv", [128, 8 * 256], BF16, kind="ExternalInput")
    wo_d = nc.dram_tensor("wo", [128, 8 * D], BF16, kind="ExternalInput")
    bq0_d = nc.dram_tensor("bq0", [128, 1], F32, kind="ExternalInput")
    bq1_d = nc.dram_tensor("bq1", [128, 1], F32, kind="ExternalInput")
    bk0_d = nc.dram_tensor("bk0", [128, 1], F32, kind="ExternalInput")
    bk1_d = nc.dram_tensor("bk1", [128, 1], F32, kind="ExternalInput")
    bvr_d = nc.dram_tensor("bvr", [1, 256], BF16, kind="ExternalInput")
    cos_d = nc.dram_tensor("cos32", [32, S], BF16, kind="ExternalInput")
    sin_d = nc.dram_tensor("sin32", [32, S], BF16, kind="ExternalInput")
    tri_d = nc.dram_tensor("tri", [128, 128], BF16, kind="ExternalInput")
    out_d = nc.dram_tensor("out", [NQB * 128, D], BF16, kind="ExternalOutput")

    recip_d = nc.dram_tensor("recipd", [4 * HPC, 512], BF16, kind="Internal")
    a2ain_d = [
        nc.dram_tensor(f"a2ain{qb}", [N_CORES * 128, 128], BF16, kind="Internal")
        for qb in range(3)
    ]
    a2aout_d = [
        nc.dram_tensor(f"a2aout{qb}", [N_CORES * 128, 128], BF16, kind="Internal")
        for qb in range(3)
    ]
    # qb3 ships per head-pair wave in two half-width A2As: wave 0's
    # collective (and its rendezvous skew) hides under wave 1's attention
    a2ain3_d = [
        nc.dram_tensor(f"a2ain3{w}", [N_CORES * 128, 64], BF16, kind="Internal")
        for w in range(2)
    ]
    a2aout3_d = [
        nc.dram_tensor(f"a2aout3{w}", [N_CORES * 128, 64], BF16, kind="Internal")
        for w in range(2)
    ]

    with tile.TileContext(nc) as tc:
        with (
            tc.tile_pool(name="const", bufs=1) as cpool,
            tc.tile_pool(name="xp", bufs=1) as xpool,
            tc.tile_pool(name="qk", bufs=1) as qkpool,
            tc.tile_pool(name="rtmp", bufs=2) as rtmp,
            tc.tile_pool(name="ptp", bufs=3) as ptpool,
            tc.tile_pool(name="att", bufs=1) as attpool,
            tc.tile_pool(name="bc", bufs=2) as bcpool,
            tc.tile_pool(name="osb", bufs=2) as opool,
            tc.tile_pool(name="a4p", bufs=4) as a4pool,
        ):
            # ---- load order tuned for earliest first matmul ----
            # wq0 + per-dt x chunks first so the q0/st0 matmul stream starts
            # as soon as ~1 chunk lands; small constants ride the scalar /
            # gpsimd DGEs so they don't serialize behind the bulk loads on
            # the sync DGE.
            def load_w(dram, cols):
                t = cpool.tile([128, 8, cols], BF16, tag=f"w_{dram.name}")
                nc.sync.dma_start(
                    t[:].rearrange("p a b -> p (a b)"), dram.ap()
                )
                return t

            def load_c(dram, shape, dt, tag, eng):
                t = cpool.tile(shape, dt, tag=tag)
                eng.dma_start(t[:], dram.ap())
                return t

            xts_r = xts_d.ap().rearrange(
                "(st p) (dt s) -> st p dt s", st=4, dt=8
            )

            wq0_sb = load_w(wq0_d, 128)
            x0d = [
                xpool.tile([128, 512], BF16, tag=f"x0d{dt}", name=f"x0d{dt}")
                for dt in range(8)
            ]
            for dt in range(8):
                nc.sync.dma_start(x0d[dt][:], xts_r[0][:, dt, :])
            wq1_sb = load_w(wq1_d, 128)
            wk0_sb = load_w(wk0_d, 128)
            wk1_sb = load_w(wk1_d, 128)
            xt1_sb = xpool.tile([128, 8, 512], BF16, tag="xt1")
            nc.sync.dma_start(xt1_sb[:], xts_r[1])
            wv_sb = load_w(wv_d, 256)
            xt2_sb = xpool.tile([128, 8, 512], BF16, tag="xt2")
            nc.sync.dma_start(xt2_sb[:], xts_r[2])
            xt3_sb = xpool.tile([128, 8, 512], BF16, tag="xt3")
            nc.sync.dma_start(xt3_sb[:], xts_r[3])

            bq0_sb = load_c(bq0_d, [128, 1], F32, "bq0", nc.scalar)
            bq1_sb = load_c(bq1_d, [128, 1], F32, "bq1", nc.scalar)
            bk0_sb = load_c(bk0_d, [128, 1], F32, "bk0", nc.scalar)
            bk1_sb = load_c(bk1_d, [128, 1], F32, "bk1", nc.scalar)
            # v-bias broadcast across all partitions: the bias-add rides the
            # V PSUM drain on DVE instead of a rank-1 matmul per block
            bvr_sb = cpool.tile([128, 4, 64], BF16, tag="bvr")
            nc.scalar.dma_start(
                bvr_sb[:].rearrange("p a b -> p (a b)"),
                bvr_d.ap().to_broadcast((128, 256)),
            )
            tri_sb = load_c(tri_d, [128, 128], BF16, "tri", nc.gpsimd)

            cos_sb = cpool.tile([128, S], BF16, tag="cos4")
            sin_sb = cpool.tile([128, S], BF16, tag="sin4")
            nc.gpsimd.dma_start(cos_sb[0:32, :], cos_d.ap())
            nc.gpsimd.dma_start(sin_sb[0:32, :], sin_d.ap())
            nc.gpsimd.dma_start(cos_sb[32:64, :], cos_sb[0:32, :])
            nc.gpsimd.dma_start(sin_sb[32:64, :], sin_sb[0:32, :])
            nc.gpsimd.dma_start(cos_sb[64:128, :], cos_sb[0:64, :])
            nc.gpsimd.dma_start(sin_sb[64:128, :], sin_sb[0:64, :])

            def xt_ap(st, dt):
                """x^T slice for (st, dt): [128, 512] local columns."""
                if st == 0:
                    return x0d[dt][:]
                return (xt1_sb, xt2_sb, xt3_sb)[st - 1][:, dt, :]

            # ---- persistent SBUF state ----
            q0_sb = qkpool.tile([128, S], BF16, tag="q0")
            q1_sb = qkpool.tile([128, S], BF16, tag="q1")
            k0_sb = qkpool.tile([128, S], BF16, tag="k0")
            k1_sb = qkpool.tile([128, S], BF16, tag="k1")
            v_sb = qkpool.tile([128, NKT, HPC * 65], BF16, tag="v")
            nc.vector.memset(
                v_sb[:].rearrange("p t (h c) -> p t h c", c=65)[:, :, :, 64:65], 1.0
            )
            qm = [
                qkpool.tile([128, S], BF16, tag=f"qm{w}", name=f"qm{w}")
                for w in range(2)
            ]
            km = [
                qkpool.tile([128, S], BF16, tag=f"km{w}", name=f"km{w}")
                for w in range(2)
            ]

            sums_sb = attpool.tile([128, 64], F32, tag="sums")
            recip_sb = attpool.tile([128, 64], BF16, tag="recip")
            tri_b2 = tri_sb[:, None, :].to_broadcast((128, 2, 128))

            attTs = []
            last_pv = {}

            # ---- helpers ----
            def proj_st(projp, st):
                sl = slice(st * 512, (st + 1) * 512)
                for dst, w_sb, b_sb in (
                    (q0_sb, wq0_sb, bq0_sb),
                    (q1_sb, wq1_sb, bq1_sb),
                    (k0_sb, wk0_sb, bk0_sb),
                    (k1_sb, wk1_sb, bk1_sb),
                ):
                    ps = projp.tile([128, 512], F32, tag="pq")
                    for dt_ in range(8):
                        nc.tensor.matmul(
                            ps[:],
                            w_sb[:, dt_, :],
                            xt_ap(st, dt_),
                            start=(dt_ == 0),
                            stop=(dt_ == 7),
                        )
                    nc.vector.tensor_scalar(
                        dst[:, sl], ps[:], b_sb[:, 0:1], None, ALU.add
                    )
                if st == 0:
                    warm_act = cpool.tile([128, 1], F32, tag="warm_act")
                    nc.scalar.activation(warm_act[:], bq0_sb[:], AF.Exp)
                for sub in range(4):
                    stv = 4 * st + sub
                    ps = projp.tile([128, 256], F32, tag="pvj")
                    for dt_ in range(8):
                        nc.tensor.matmul(
                            ps[:],
                            xt_ap(st, dt_)[:, sub * 128 : (sub + 1) * 128],
                            wv_sb[:, dt_, :],
                            start=(dt_ == 0),
                            stop=(dt_ == 7),
                        )
                    nc.vector.tensor_tensor(
                        v_sb[:, stv, :].rearrange("p (h c) -> p h c", c=65)[
                            :, :, 0:64
                        ],
                        ps[:].rearrange("p (h c) -> p h c", c=64),
                        bvr_sb[:],
                        ALU.add,
                    )
                # RoPE for this st (DVE)
                for x0, x1 in ((q0_sb, q1_sb), (k0_sb, k1_sb)):
                    m1 = rtmp.tile([128, 512], BF16, tag="m1")
                    m2 = rtmp.tile([128, 512], BF16, tag="m2")
                    m3 = rtmp.tile([128, 512], BF16, tag="m3")
                    m4 = rtmp.tile([128, 512], BF16, tag="m4")
                    nc.vector.tensor_tensor(m1[:], x0[:, sl], cos_sb[:, sl], ALU.mult)
                    nc.vector.tensor_tensor(m2[:], x1[:, sl], sin_sb[:, sl], ALU.mult)
                    nc.vector.tensor_tensor(m3[:], x0[:, sl], sin_sb[:, sl], ALU.mult)
                    nc.vector.tensor_tensor(m4[:], x1[:, sl], cos_sb[:, sl], ALU.mult)
                    nc.vector.tensor_tensor(x0[:, sl], m1[:], m2[:], ALU.subtract)
                    nc.vector.tensor_tensor(x1[:, sl], m3[:], m4[:], ALU.add)
                # merge RoPE'd halves into per-head-contiguous layouts.
                # All merges ride the sync DGE: the gpsimd queue blocks on
                # each collective's completion, and k-merges queued behind a
                # trigger would stall the next q-block's scores on the
                # previous A2A's rendezvous skew.  The first scores of a
                # block only need the first ~4 merges, so the serial chain
                # is demand-paced.
                for w in range(2):
                    for hh in range(2):
                        h = 2 * w + hh
                        nc.sync.dma_start(
                            qm[w][64 * hh : 64 * hh + 32, sl],
                            q0_sb[32 * h : 32 * h + 32, sl],
                        )
                        nc.sync.dma_start(
                            qm[w][64 * hh + 32 : 64 * hh + 64, sl],
                            q1_sb[32 * h : 32 * h + 32, sl],
                        )
                        nc.sync.dma_start(
                            km[w][64 * hh : 64 * hh + 32, sl],
                            k0_sb[32 * h : 32 * h + 32, sl],
                        )
                        nc.sync.dma_start(
                            km[w][64 * hh + 32 : 64 * hh + 64, sl],
                            k1_sb[32 * h : 32 * h + 32, sl],
                        )

            def drain_pair(stg, att4, w, pvl):
                """Copy a head-pair's denominator rows + attended blocks out
                of PSUM into the staging tiles (releases the PSUM banks)."""
                nc.vector.tensor_copy(
                    stg[64:65, 2 * w : 2 * w + 2, :], pvl[64:65, :, :]
                )
                nc.vector.tensor_copy(
                    att4[:, 2 * w : 2 * w + 2, :], pvl[0:64, :, :]
                )

            ship_st = {}
            ship_sums = {}

            def ship_a(qb, stg, att4):
                """Ship stage A (emitted right after qb's drains):
                denominators -> reciprocals -> DRAM -> broadcast loads."""
                ship_sums[qb] = nc.sync.dma_start(
                    sums_sb[32 * qb : 32 * qb + 32, :],
                    stg[64:65, :, :],
                )
                with nc.allow_low_precision(
                    reason="bf16 recip matches the prior rb-cast path"
                ):
                    nc.vector.reciprocal(
                        recip_sb[32 * qb : 32 * qb + 32, :],
                        sums_sb[32 * qb : 32 * qb + 32, :],
                    )
                nc.sync.dma_start(
                    recip_d[4 * qb : 4 * qb + 4, :],
                    recip_sb[32 * qb : 32 * qb + 32, :],
                )
                # all broadcast loads on sync: the gpsimd queue blocks on the
                # previous collective's completion, so anything here routed
                # through it inherits that collective's rendezvous skew
                bct4 = bcpool.tile([64, HPC, 512], BF16, tag="bct4",
                                   name=f"bct4_{qb}")
                for h in range(HPC):
                    nc.sync.dma_start(
                        bct4[:, h, :],
                        recip_d[4 * qb + h : 4 * qb + h + 1, :]
                        .to_broadcast((64, 512)),
                    )
                ship_st[qb] = (att4, bct4)

            def ship_b(qb, pin_after=None):
                """Ship stage B: normalize, scatter into the A2A layout, and
                launch the collective.  Deferred one pipeline phase after
                ship_a so each engine reaches these ops with the inputs long
                since ready -- a stage-A DMA latency here would head-of-line
                block the engine FIFOs and cascade into rendezvous skew.
                (Gathers from a2aout are deferred even further, right before
                the consuming out-projection.)"""
                att4, bct4 = ship_st.pop(qb)
                a2a_w = a2ain_d[qb].ap().rearrange(
                    "(j hp c) (t r) -> hp t c j r", j=8, hp=2, c=64, t=2
                )
                for t in range(2):
                    sl_ = att4[:, 2 * t : 2 * t + 2, :].rearrange(
                        "c h r -> c (h r)"
                    )
                    nc.vector.tensor_tensor(
                        sl_,
                        sl_,
                        bct4[:, 2 * t : 2 * t + 2, :].rearrange(
                            "c h r -> c (h r)"
                        ),
                        ALU.mult,
                    )
                    for i in range(2):
                        h = 2 * t + i
                        nc.sync.dma_start(
                            a2a_w[h % 2, h // 2],
                            att4[:, h, :].rearrange("c (j r) -> c j r", j=8),
                        )
                cc = nc.gpsimd.collective_compute(
                    "AllToAll",
                    ALU.bypass,
                    replica_groups=A2A_GROUP,
                    ins=[a2ain_d[qb][:]],
                    outs=[a2aout_d[qb][:]],
                )
                if pin_after is not None:
                    # the collective blocks its queue until completion; make
                    # sure earlier gathers cannot be scheduled behind it
                    tile.add_dep_helper(
                        cc.ins, pin_after.ins, sync=True,
                        reason="collective after prior gathers",
                    )

            ship3_sums = {}

            def ship3_wave(w, stg, att4):
                """qb3 ships per wave: sums -> recip -> broadcast -> norm ->
                half-width A2A over this head-pair's columns.  Wave 0's
                collective runs under wave 1's attention; only wave 1's is
                tail-exposed."""
                qb = 3
                # 32-aligned 32x32 staging region per wave (wave 1 reuses
                # qb0's long-retired rows)
                r0 = 96 if w == 0 else 0
                ship3_sums[w] = nc.sync.dma_start(
                    sums_sb[r0 : r0 + 32, 0:32],
                    stg[64:65, 2 * w : 2 * w + 2, :],
                )
                with nc.allow_low_precision(
                    reason="bf16 recip matches the prior rb-cast path"
                ):
                    nc.vector.reciprocal(
                        recip_sb[r0 : r0 + 32, 0:32],
                        sums_sb[r0 : r0 + 32, 0:32],
                    )
                nc.sync.dma_start(
                    recip_d[4 * qb + 2 * w : 4 * qb + 2 * w + 2, :],
                    recip_sb[r0 : r0 + 32, 0:32],
                )
                bct2 = bcpool.tile([64, 2, 512], BF16, tag="bct2",
                                   name=f"bct2_{w}")
                for i in range(2):
                    nc.sync.dma_start(
                        bct2[:, i, :],
                        recip_d[4 * qb + 2 * w + i : 4 * qb + 2 * w + i + 1, :]
                        .to_broadcast((64, 512)),
                    )
                sl_ = att4[:, 2 * w : 2 * w + 2, :].rearrange(
                    "c h r -> c (h r)"
                )
                nc.vector.tensor_tensor(
                    sl_, sl_, bct2[:].rearrange("c h r -> c (h r)"), ALU.mult
                )
                a2a_w = a2ain3_d[w].ap().rearrange(
                    "(j hp c) r -> hp c j r", j=8, hp=2, c=64
                )
                for i in range(2):
                    nc.sync.dma_start(
                        a2a_w[i],
                        att4[:, 2 * w + i, :].rearrange(
                            "c (j r) -> c j r", j=8
                        ),
                    )
                nc.gpsimd.collective_compute(
                    "AllToAll",
                    ALU.bypass,
                    replica_groups=A2A_GROUP,
                    ins=[a2ain3_d[w][:]],
                    outs=[a2aout3_d[w][:]],
                )

            def gather3_wave(w, attT):
                """Gather wave w's redistributed rows into attT slots
                4w..4w+4.  Wave 0: gpsimd only (it is blocked on the wave-0
                collective anyway, hidden under wave 1).  Wave 1: gpsimd +
                scalar (both idle in the tail)."""
                srcr = a2aout3_d[w].ap().rearrange(
                    "(i p) r -> p i r", p=128
                )
                for bh in range(2):
                    eng = nc.gpsimd if w == 0 else (nc.gpsimd, nc.scalar)[bh]
                    eng.dma_start(
                        attT[:, 4 * w : 4 * w + 4, 64 * bh : 64 * bh + 64],
                        srcr[:, 4 * bh : 4 * bh + 4, :],
                    )

            def gather_attT(qb, tail=False):
                """Pull this q-block's redistributed attended rows out of the
                A2A output.  These wait on the collective, so they must never
                sit ahead of other pending work in a DGE queue; the final
                gather goes gpsimd-only so the sync queue (y stores) never
                blocks on the last collective."""
                attT = a4pool.tile([128, 8, 128], BF16, tag="attT",
                                   name=f"attT_{qb}")
                srcr = a2aout_d[qb].ap().rearrange(
                    "(i p) (t r) -> p i t r", p=128, t=2
                )
                last = None
                for t in range(2):
                    for bh in range(2):
                        # tail: scalar's queue is empty by now, so blocking
                        # it on the last collective is free parallelism
                        eng = (nc.gpsimd, nc.scalar)[bh] if tail else (
                            (nc.gpsimd, nc.sync)[bh]
                        )
                        last = eng.dma_start(
                            attT[:, 4 * t : 4 * t + 4, 64 * bh : 64 * bh + 64],
                            srcr[:, 4 * bh : 4 * bh + 4, t],
                        )
                attTs.append(attT)
                return last

            def qb_lite(plite, qb, psc_bufs=2, paired=False, ship3=False):
                """Attention for qb in two head-pair passes (runs interleaved
                with the projection phase).  paired=True packs the two heads
                of a wave into one 2-bank psc tile: the scores matmuls run
                concurrently via tile_position row strips and the exp is one
                batched ACT call -- needs 4+2 banks, only affordable for the
                standalone qb3 block."""
                stg = bcpool.tile([65, HPC, 512], F32, tag="stg",
                                  name=f"stg_{qb}")
                att4 = bcpool.tile([64, HPC, 512], BF16, tag="att4",
                                   name=f"att4_{qb}")
                nkt = 4 * qb + 4
                for w in range(2):
                    pvl = plite.tile([65, 2, 512], F32, tag="pvl",
                                     name=f"pvl_{qb}_{w}")
                    for kt in range(nkt):
                        j = kt - 4 * qb
                        qlo = max(0, j * 128)
                        g0 = qb * 512 + qlo
                        g1 = (qb + 1) * 512
                        if paired:
                            psc = plite.tile(
                                [128, 2, 512], F32, tag="psc2",
                                bufs=psc_bufs,
                                name=f"psc2_{qb}_{w}_{kt}",
                            )
                            for hh in range(2):
                                nc.tensor.matmul(
                                    psc[:, hh, qlo:512],
                                    km[w][64 * hh : 64 * hh + 64,
                                          kt * 128 : (kt + 1) * 128],
                                    qm[w][64 * hh : 64 * hh + 64, g0:g1],
                                    start=True,
                                    stop=True,
                                    tile_position=(64 * hh, 0),
                                )
                            pt = ptpool.tile(
                                [128, 2, 512], BF16, tag="ptp2",
                                name=f"ptp2_{qb}_{w}_{kt}",
                            )
                            if qlo == 0:
                                nc.scalar.activation(
                                    pt[:].rearrange("p a b -> p (a b)"),
                                    psc[:].rearrange("p a b -> p (a b)"),
                                    AF.Exp, scale=0.125,
                                )
                            else:
                                nc.scalar.activation(
                                    pt[:, :, qlo:512], psc[:, :, qlo:512],
                                    AF.Exp, scale=0.125,
                                )
                            if g0 == kt * 128:
                                nc.vector.tensor_tensor(
                                    pt[:, :, qlo : qlo + 128],
                                    pt[:, :, qlo : qlo + 128],
                                    tri_b2,
                                    ALU.mult,
                                )
                            for hh in range(2):
                                h = 2 * w + hh
                                mm = nc.tensor.matmul(
                                    pvl[:, hh, qlo:512],
                                    v_sb[:, kt, 65 * h : 65 * h + 65],
                                    pt[:, hh, qlo:512],
                                    start=(kt == 0),
                                    stop=(kt == nkt - 1),
                                )
                                last_pv[qb] = mm
                            continue
                        # both heads' scores emitted adjacently: different
                        # row strips + different PSUM banks, so the PE packs
                        # them concurrently (second MM is ~4ns)
                        pscs = []
                        for hh in range(2):
                            psc = plite.tile(
                                [128, 512], F32, tag="pscl", bufs=psc_bufs,
                                name=f"pscl_{qb}_{w}_{kt}_{hh}",
                            )
                            nc.tensor.matmul(
                                psc[:, qlo:512],
                                km[w][64 * hh : 64 * hh + 64,
                                      kt * 128 : (kt + 1) * 128],
                                qm[w][64 * hh : 64 * hh + 64, g0:g1],
                                start=True,
                                stop=True,
                                tile_position=(64 * hh, 0),
                            )
                            pscs.append(psc)
                        for hh in range(2):
                            h = 2 * w + hh
                            pt = ptpool.tile(
                                [128, 512], BF16, tag="ptl",
                                name=f"ptl_{qb}_{w}_{kt}_{hh}",
                            )
                            nc.scalar.activation(
                                pt[:, qlo:512], pscs[hh][:, qlo:512],
                                AF.Exp, scale=0.125,
                            )
                            if g0 == kt * 128:
                                nc.vector.tensor_tensor(
                                    pt[:, qlo : qlo + 128],
                                    pt[:, qlo : qlo + 128],
                                    tri_sb[:],
                                    ALU.mult,
                                )
                            mm = nc.tensor.matmul(
                                pvl[:, hh, qlo:512],
                                v_sb[:, kt, 65 * h : 65 * h + 65],
                                pt[:, qlo:512],
                                start=(kt == 0),
                                stop=(kt == nkt - 1),
                            )
                            last_pv[qb] = mm
                    drain_pair(stg, att4, w, pvl)
                    if ship3:
                        ship3_wave(w, stg, att4)
                        gather3_wave(w, attT3)
                if not ship3:
                    ship_a(qb, stg, att4)

            # ---- projection phase with qb0-qb2 lite attention woven in ----
            with (
                tc.tile_pool(name="projp", bufs=2, space="PSUM") as projp,
                tc.tile_pool(name="plite", bufs=1, space="PSUM") as plite,
            ):
                proj_st(projp, 0)
                qb_lite(plite, 0)
                proj_st(projp, 1)
                ship_b(0)
                qb_lite(plite, 1)
                proj_st(projp, 2)
                ship_b(1)
                qb_lite(plite, 2)
                proj_st(projp, 3)
                ship_b(2)
                # gathers for qb0-2 land here, mid-body: their collectives
                # complete during the projection phases, so these never
                # block, and attT0-2 are ready before the qb3 filler window
                for qb in range(3):
                    gather_attT(qb)

            # wo arrives during attention; needed only for the tail out-proj
            wo_sb = cpool.tile([128, 8, D], BF16)
            nc.sync.dma_start(wo_sb[:].rearrange("p a b -> p (a b)"), wo_d.ap())

            # ---- full-width attention for qb3, with the out-projections of
            # qb0-2 emitted after it as lower-priority PE gap-filler: they
            # hide the exp->PV chain latency, keep HAM warm through qb3's
            # ship window, and leave only qb3's own out-proj in the tail.
            def outproj(opsum, qb, after=None):
                attT = attTs[qb]
                y_sb = opool.tile([128, D], BF16, tag="y", name=f"y_{qb}")
                last = None
                for dc in range(2):
                    po = opsum.tile([128, 512], F32, tag="po",
                                    name=f"po_{qb}_{dc}")
                    for s in range(8):
                        mm = nc.tensor.matmul(
                            po[:],
                            attT[:, s, :],
                            wo_sb[:, s, dc * 512 : (dc + 1) * 512],
                            start=(s == 0),
                            stop=(s == 7),
                        )
                        last = mm
                        if after is not None and dc == 0 and s == 0:
                            # explicit ordering handle (scheduler otherwise
                            # reorders these across the ship3 window)
                            tile.add_dep_helper(
                                mm.ins, after.ins, sync=True,
                                reason="outproj ordering across ship3",
                            )
                    nc.vector.tensor_copy(
                        y_sb[:, dc * 512 : (dc + 1) * 512], po[:]
                    )
                nc.sync.dma_start(out_d[qb * 128 : (qb + 1) * 128, :], y_sb[:])
                return last

            # qb3 runs lite-style with a deep psc rotation (4 banks) so the
            # scores stream can run ahead of the ACT exp pacing; out-proj of
            # qb0/1 interleaves as PE filler during qb3's attention, out-proj
            # of qb2 is held back to bridge the ship3/A2A window, and
            # out-proj 3 is pinned after it so the scheduler cannot park
            # out-proj 2 behind the gather-gated final block.
            with (
                tc.tile_pool(name="pl3", bufs=1, space="PSUM") as pl3,
                tc.tile_pool(name="op", bufs=2, space="PSUM") as opsum,
            ):
                attT3 = a4pool.tile([128, 8, 128], BF16, tag="attT",
                                    name="attT_3")
                attTs.append(attT3)
                qb_lite(pl3, 3, psc_bufs=2, paired=True, ship3=True)
                outproj(opsum, 0)
                outproj(opsum, 1, after=ship3_sums[1])
                op2_last = outproj(opsum, 2, after=ship3_sums[1])
                outproj(opsum, 3, after=op2_last)

    nc.compile()
    return nc


def make_in_maps(x, Wq, bq, Wk, bk, Wv, bv, Wo):
    inv = 1.0 / (ROPE_BASE ** (2.0 * np.arange(32, dtype=np.float64) / L))
    ang = np.arange(S, dtype=np.float64)[:, None] * inv[None, :]  # [S, 32]
    cos32 = np.ascontiguousarray(np.cos(ang).T).astype(NPBF16)  # [32, S]
    sin32 = np.ascontiguousarray(np.sin(ang).T).astype(NPBF16)
    tri = (np.arange(128)[None, :] >= np.arange(128)[:, None]).astype(NPBF16)

    # Wo rows permuted to the attT channel order: slot s = 4*t + gi, row
    # s*128+p holds Wo[64*(4*gi + 2*t + (p>=64)) + p%64]  (same for all cores).
    perm = np.empty(D, np.int64)
    for s_ in range(8):
        t, gi = divmod(s_, 4)
        for p in range(128):
            h = 4 * gi + 2 * t + (1 if p >= 64 else 0)
            perm[s_ * 128 + p] = 64 * h + (p % 64)
    wo_perm = np.ascontiguousarray(Wo[perm, :]).astype(NPBF16)
    wo_pm = np.ascontiguousarray(
        wo_perm.reshape(8, 128, D).transpose(1, 0, 2).reshape(128, 8 * D)
    )

    def _pm(w):
        # [1024, C] -> partition-major [128, 8*C] bf16 (contiguous lines)
        w = np.asarray(w).astype(NPBF16)
        c = w.shape[1]
        return np.ascontiguousarray(
            w.reshape(8, 128, c).transpose(1, 0, 2).reshape(128, 8 * c)
        )

    in_maps = []
    for c in range(N_CORES):
        b, g = divmod(c, HPC)
        even = np.concatenate([64 * h + 2 * np.arange(32) for h in range(4 * g, 4 * g + 4)])
        odd = even + 1
        vcols = np.arange(256 * g, 256 * (g + 1))
        xt = np.ascontiguousarray(x[b].T).astype(NPBF16)  # [D, S]
        xts = np.ascontiguousarray(
            xt.reshape(8, 128, 4, 512).transpose(2, 1, 0, 3).reshape(512, 4096)
        )
        in_maps.append(
            {
                "xts": xts,
                "wq0": _pm(Wq[:, even]),
                "wq1": _pm(Wq[:, odd]),
                "wk0": _pm(Wk[:, even]),
                "wk1": _pm(Wk[:, odd]),
                "wv": _pm(Wv[:, vcols]),
                "wo": wo_pm,
                "bq0": bq[even].reshape(128, 1).astype(np.float32),
                "bq1": bq[odd].reshape(128, 1).astype(np.float32),
                "bk0": bk[even].reshape(128, 1).astype(np.float32),
                "bk1": bk[odd].reshape(128, 1).astype(np.float32),
                "bvr": bv[vcols].reshape(1, 256).astype(NPBF16),
                "cos32": cos32,
                "sin32": sin32,
                "tri": tri,
            }
        )
    return in_maps


def assemble_output(results, bo):
    out = np.empty((B, S, D), np.float32)
    for c in range(N_CORES):
        sh = np.asarray(results[c]["out"]).astype(np.float32).reshape(NQB, 128, D)
        for qb in range(NQB):
            r0 = qb * 512 + c * 64
            out[0, r0 : r0 + 64, :] = sh[qb][0:64]
            out[1, r0 : r0 + 64, :] = sh[qb][64:128]
    out += bo[None, None, :].astype(np.float32)
    return out


_CACHE = {}


def kernel(x, Wq, bq, Wk, bk, Wv, bv, Wo, bo, **run_kwargs):
    if "nc" not in _CACHE:
        _CACHE["nc"] = build_program()
    nc = _CACHE["nc"]
    in_maps = make_in_maps(
        np.asarray(x), np.asarray(Wq), np.asarray(bq), np.asarray(Wk),
        np.asarray(bk), np.asarray(Wv), np.asarray(bv), np.asarray(Wo),
    )
    res = bass_utils.run_bass_kernel_spmd(
        nc, in_maps, core_ids=list(range(N_CORES)), **run_kwargs
    )
    out = assemble_output(res.results, np.asarray(bo))
    kernel.last_results = res
    return out

